# revision 5
# baseline (speedup 1.0000x reference)
"""DeepAndWide Trainium2 kernel (8 NeuronCores, SPMD via Bass/Tile).

Math:
    g = concat(hs, ht, 1) @ W_L.T + b_L            # [B, 2D] deep path
    v = outer(hs_b, ht_b) flattened -> [B, D*D]    # wide-path features
    u = v @ W_L2.T + b_L2                          # [B, 2D]
    out = concat(g, u, 1)                          # [B, 4D]

Sharding: W_L2 column-sharded over the D*D contraction dim across 8 cores
(each core owns 48 outer-product rows i); every core computes a partial u
over the full batch; the host sums the partials. g is data-parallel over
batch (128 rows per core).

v2 structure (vs v1):
  * Full-contraction PSUM accumulation: 6 PSUM banks hold uT[oc] for one
    batch half (bh) across the whole local contraction; no SBUF fp32
    accumulator, no per-group DVE adds. Weights are re-streamed per bh
    (2x weight DMA, still well under the PE-bound span).
  * fp8 DoubleRow for the last NI8 of 48 i's: W slice pre-scaled x16 and
    quantized e4m3 on host; hs slice pre-scaled 1/16 so the DVE-built
    vT tiles land in e4m3 range. One DoubleRow matmul contracts 2 k-tiles
    per 512-col stream (~1.4-1.8x the fp16 rate). Rel-err budget measured
    offline: alpha=0.25 -> ~1.6e-2 (< 2e-2 gate).
  * Dedicated engine queues (no head-of-line blocking): sync=weights,
    scalar=htt/xt/wlt DMA + all PSUM->SBUF flush copies, gpsimd=hsb +
    bh0/g outputs, vector=vT builds only, tensor=matmuls.
  * Staggered per-oc flush at each bh end; bh0 outputs DMA during bh1
    compute; deep path g slotted mid-bh0.
"""

import os as _os

import numpy as np

import concourse.bass as bass
import concourse.mybir as mybir
from concourse import tile
from concourse.bass_utils import run_bass_kernel_spmd

B = 1024
D = 384
NCORES = 8
IC = D // NCORES          # 48 outer-product rows (i) per core
KC = IC * D               # 18432 contraction columns per core
TWO_D = 2 * D             # 768

NI8 = int(_os.environ.get("KERNEL_NI8", "14"))   # fp8 i's per core (even)
assert NI8 % 2 == 0 and 0 <= NI8 <= IC
NI16 = IC - NI8           # fp16 i's per core
NT16 = NI16 * 3           # fp16 k-tiles (128 rows each)
NP8 = (NI8 * 3) // 2      # fp8 DoubleRow pairs (2 k-tiles each)
W8SCALE = 16.0            # host: W8 = W*16, hs8 = hs/16 (product scale = 1)

WB = 6                    # fp16 k-tiles per weight DMA batch
PB = 6                    # fp8 pairs per weight DMA batch

F32 = mybir.dt.float32
F16 = mybir.dt.float16
F8 = mybir.dt.float8e4
NP_F16 = np.float16
NP_F8 = mybir.dt.np(F8)
DR = mybir.MatmulPerfMode.DoubleRow

LAST_EXEC_TIME_NS = None
LAST_RESULTS = None


def _split_excess_waits(nc):
    """walrus rejects >1 sync-wait on several instruction structs (fp32/f32r
    Matmult, Drain, ...). Hoist all but the last wait of any multi-wait
    instruction onto single-wait EventSemaphore instructions inserted just
    before it on the same engine."""
    n = [0]

    def fresh():
        n[0] += 1
        return f"WSPLIT-{n[0]}"

    for f in nc.m.functions:
        for blk in f.blocks:
            out = []
            changed = False
            for ins in blk.instructions:
                si = ins.sync_info
                if si is not None and len(si.on_wait) > 1:
                    waits = list(si.on_wait)
                    for w in waits[:-1]:
                        ev = mybir.InstEventSemaphore(
                            name=fresh(),
                            engine=ins.engine,
                            ins=[],
                            outs=[],
                            sync_info=mybir.SyncInfo(on_wait=[w], on_update=[]),
                        )
                        out.append(ev)
                    ins.sync_info = mybir.SyncInfo(
                        on_wait=[waits[-1]], on_update=list(si.on_update)
                    )
                    changed = True
                out.append(ins)
            if changed:
                blk.instructions = out


def _strip_unused_mm_incs(nc):
    """Every matmul carries a +1 update on the PE semaphore; the EVT_SEM
    register write costs the PE queue ~26ns each. Keep only the increments
    whose cumulative tick some wait actually references (plus the final
    one), and renumber all waits on that semaphore accordingly."""
    from collections import defaultdict

    for f in nc.m.functions:
        upd_insts = defaultdict(list)
        wait_refs = defaultdict(list)
        for blk in f.blocks:
            for ins in blk.instructions:
                si = ins.sync_info
                if not si:
                    continue
                for u in si.on_update:
                    upd_insts[u.id].append((ins, u))
                for w in si.on_wait:
                    wait_refs[w.id].append(w)

        for sem_id, upds in upd_insts.items():
            if not all(
                type(i).__name__ == "InstMatmult"
                and u.update_mode == "sem-inc"
                and u.update_value == 1
                for i, u in upds
            ):
                continue
            ws = wait_refs.get(sem_id, [])
            if any(
                w.wait_mode != "sem-ge-imm" or w.wait_reg is not None for w in ws
            ):
                continue
            used = {w.wait_value for w in ws}
            n = len(upds)
            keep = []
            kept_prefix = []
            kept = 0
            for tick in range(1, n + 1):
                k = tick in used or tick == n
                keep.append(k)
                kept += 1 if k else 0
                kept_prefix.append(kept)
            for (ins, u), k in zip(upds, keep):
                if not k:
                    si = ins.sync_info
                    ins.sync_info = mybir.SyncInfo(
                        on_wait=list(si.on_wait),
                        on_update=[x for x in si.on_update if x.id != sem_id],
                    )
            for w in ws:
                v = w.wait_value
                if v >= 1:
                    w.wait_value = kept_prefix[min(v, n) - 1]


def _gen():
    nc = bass.Bass()

    w2t = nc.dram_tensor("w2t", [NT16 * 128, TWO_D], F16, kind="ExternalInput")
    hsb16 = nc.dram_tensor("hsb16", [max(NI16, 1), 128, B], F16, kind="ExternalInput")
    htt = nc.dram_tensor("htt", [D, B], F16, kind="ExternalInput")
    xt = nc.dram_tensor("xt", [TWO_D, 128], F16, kind="ExternalInput")
    wlt = nc.dram_tensor("wlt", [TWO_D, TWO_D], F16, kind="ExternalInput")
    if NP8:
        w28 = nc.dram_tensor("w28", [NP8, 128, 2 * TWO_D], F8, kind="ExternalInput")
        hsb8 = nc.dram_tensor("hsb8", [NI8, 128, B], F16, kind="ExternalInput")
    u_out = nc.dram_tensor("u_out", [TWO_D, B], F32, kind="ExternalOutput")  # uT
    g_out = nc.dram_tensor("g_out", [128, TWO_D], F32, kind="ExternalOutput")

    u_out_r = u_out.rearrange("(c p) b -> c p b", p=128)          # [6,128,B]
    w2_pto = w2t.rearrange("(t p) o -> p t o", p=128)             # [128,NT16,2D]
    hsb16_pib = hsb16.rearrange("i p b -> p i b")                 # [128,NI16,B]
    htt_pjb = htt.rearrange("(j p) b -> p j b", p=128)            # [128,3,B]
    if NP8:
        w28_pnq = w28.rearrange("n p q -> p n q")                 # [128,NP8,1536]
        hsb8_pib = hsb8.rearrange("i p b -> p i b")               # [128,NI8,B]

    # fp16 weight batching: first 3 fine allocations (1,2,3 tiles) for a fast
    # ramp, then WB-tile batches. Same batch schedule reused for bh1 except
    # everything coarse.
    fine = [1, 2, 3, 3, 3]
    rest = NT16 - sum(fine)
    assert rest % WB == 0
    batches_bh0 = fine + [WB] * (rest // WB)
    batches_bh1 = [WB] * (NT16 // WB)
    p_batches = [PB] * (NP8 // PB) if NP8 else []
    if NP8 % PB:
        p_batches.insert(0, NP8 % PB)

    with tile.TileContext(nc) as tc:
        with (
            tc.tile_pool(name="const", bufs=1) as constp,
            tc.tile_pool(name="w2p", bufs=3) as w2p,
            tc.tile_pool(name="w8p", bufs=2) as w8p,
            tc.tile_pool(name="vtp", bufs=8) as vtp,
            tc.tile_pool(name="vt8p", bufs=7) as vt8p,
            tc.tile_pool(name="accp", bufs=1, space="PSUM") as accp,
            tc.tile_pool(name="gpsum", bufs=1, space="PSUM") as gps,
            tc.tile_pool(name="ustp", bufs=8) as ustp,
            tc.tile_pool(name="gpool", bufs=1) as gp,
        ):
            # ---- constants / inputs resident in SBUF ----
            # bh0's htt split so the first vT build waits only on a 128KB DMA
            htt_a = constp.tile([128, 512], F16, tag="htta")       # bh0 j0
            htt_bc = constp.tile([128, 2, 512], F16, tag="httbc")  # bh0 j1,2
            htt_d = constp.tile([128, 3, 512], F16, tag="httd")    # bh1 all j

            def htt_slice(bh, jb):
                if bh == 1:
                    return htt_d[:, jb, :]
                return htt_a[:] if jb == 0 else htt_bc[:, jb - 1, :]
            # hsb fp16: 4 single-i tiles (fine deps early), then batches of 4;
            # i0 additionally split by batch half so the very first vT build
            # waits on a 128KB DMA only.
            hsb_s0h = [
                constp.tile([128, 512], F16, tag=f"hsbs0h{h}", name=f"hsbs0h{h}")
                for h in (0, 1)
            ]
            hsb_s = [
                constp.tile([128, B], F16, tag=f"hsbs{i}", name=f"hsbs{i}")
                for i in range(1, 4)
            ]
            hb_sizes = []
            left = NI16 - 4
            while left > 0:
                hb_sizes.append(min(4, left))
                left -= 4
            n_hb = len(hb_sizes)
            hsb_b = [
                constp.tile([128, hb_sizes[n], B], F16, tag=f"hsbb{n}",
                            name=f"hsbb{n}")
                for n in range(n_hb)
            ]
            if NP8:
                hsb8_t = constp.tile([128, NI8, B], F16, tag="hsb8")
            wlt_t = gp.tile([128, 6, TWO_D], F16, tag="wlt")
            xt_t = gp.tile([128, 6, 128], F16, tag="xtt")
            g_sb = gp.tile([128, TWO_D], F32, tag="gsb")
            g_ps = gps.tile([128, TWO_D], F32, tag="gps")

            def hsb_slice(il, bh):
                bsl = slice(bh * 512, (bh + 1) * 512)
                if il == 0:
                    return hsb_s0h[bh][:]
                if il < 4:
                    return hsb_s[il - 1][:, bsl]
                n, r = divmod(il - 4, 4)
                return hsb_b[n][:, r, bsl]

            # ---- critical-path DMA issue (dedicated queues) ----
            # scalar: first-tile deps (htt j0, hs i0) then the rest of htt,
            # hs i1, and g inputs; scalar also runs the flush copies later.
            nc.sync.dma_start(hsb_s0h[0][:], hsb16[0][:, 0:512])
            nc.scalar.dma_start(htt_a[:], htt_pjb[:, 0, 0:512])
            nc.scalar.dma_start(htt_bc[:], htt_pjb[:, 1:3, 0:512])
            nc.scalar.dma_start(hsb_s0h[1][:], hsb16[0][:, 512:1024])
            nc.scalar.dma_start(hsb_s[0][:], hsb16[1])
            nc.scalar.dma_start(xt_t[:], xt.rearrange("(t p) c -> p t c", p=128))
            # gpsimd: hs i2, i3, first hs batch (needed from t=12)
            nc.gpsimd.dma_start(hsb_s[1][:], hsb16[2])
            nc.gpsimd.dma_start(hsb_s[2][:], hsb16[3])
            nc.gpsimd.dma_start(
                hsb_b[0][:], hsb16_pib[:, 4 : 4 + hb_sizes[0], :]
            )

            # sync: all wide-path weights, ring-throttled prefetch. The
            # remaining hs broadcast batches + hsb8 are interleaved into this
            # stream so they inherit its pacing instead of flooding the DMA
            # fabric up-front (which starves the PE of weights early on).
            w2_tiles = {}   # (bh, batch_idx) -> (tile, t0, nt)
            for bh in (0, 1):
                t0 = 0
                nb6 = 0
                for bi, nt in enumerate(batches_bh0 if bh == 0 else batches_bh1):
                    w2_t = w2p.tile([128, nt, TWO_D], F16, tag="w2")
                    nc.sync.dma_start(w2_t[:], w2_pto[:, t0 : t0 + nt, :])
                    w2_tiles[(bh, bi)] = (w2_t, t0, nt)
                    t0 += nt
                    if bh == 0 and nt == WB:
                        nb6 += 1
                        if nb6 == 4:
                            nc.sync.dma_start(
                                wlt_t[:],
                                wlt.rearrange("(t p) o -> p t o", p=128),
                            )
                        if nb6 == 6:
                            nc.sync.dma_start(
                                htt_d[:], htt_pjb[:, :, 512:1024]
                            )
                        if nb6 % 2 == 1 and (nb6 + 1) // 2 < n_hb:
                            n = (nb6 + 1) // 2
                            nc.sync.dma_start(
                                hsb_b[n][:],
                                hsb16_pib[:, 4 + 4 * n : 4 + 4 * n + hb_sizes[n], :],
                            )
                        if NP8 and nb6 == 10:
                            nc.sync.dma_start(hsb8_t[:], hsb8_pib[:])
                if NP8:
                    p0 = 0
                    for pi, npb in enumerate(p_batches):
                        w8_t = w8p.tile([128, npb, 2, TWO_D], F8, tag="w8")
                        nc.sync.dma_start(
                            w8_t.rearrange("p n two o -> p n (two o)"),
                            w28_pnq[:, p0 : p0 + npb, :],
                        )
                        w2_tiles[("f8", bh, pi)] = (w8_t, p0, npb)
                        p0 += npb

            # ---- main loops ----
            NACC = NT16 + (1 if NP8 else 0)  # accumulation "steps" per bank
            for bh in (0, 1):
                bsl = slice(bh * 512, (bh + 1) * 512)
                acc = [
                    accp.tile([128, 512], F32, tag=f"acc{oc}", name=f"acc{oc}")
                    for oc in range(6)
                ]

                # fp16 k-tiles
                t = 0
                for bi, _ in enumerate(batches_bh0 if bh == 0 else batches_bh1):
                    w2_t, t0, nt = w2_tiles[(bh, bi)]
                    for tt in range(nt):
                        il, jb = divmod(t, 3)
                        vt = vtp.tile([128, 512], F16, tag="vt")
                        nc.vector.tensor_mul(
                            vt[:], htt_slice(bh, jb), hsb_slice(il, bh)
                        )
                        for oc in range(6):
                            nc.tensor.matmul(
                                acc[oc][:],
                                w2_t[:, tt, oc * 128 : (oc + 1) * 128],
                                vt[:],
                                start=(t == 0),
                                stop=(t == NT16 - 1 and not NP8),
                            )
                        t += 1

                        # deep path: slotted mid-bh0 (inputs arrived long ago;
                        # PE warm; keeps both kernel head and tail clear)
                        if bh == 0 and t == 36:
                            for gt in range(6):
                                nc.tensor.matmul(
                                    g_ps[:, 0:512],
                                    xt_t[:, gt, :],
                                    wlt_t[:, gt, 0:512],
                                    start=(gt == 0),
                                    stop=(gt == 5),
                                )
                                nc.tensor.matmul(
                                    g_ps[:, 512:768],
                                    xt_t[:, gt, :],
                                    wlt_t[:, gt, 512:768],
                                    start=(gt == 0),
                                    stop=(gt == 5),
                                )
                            nc.scalar.copy(g_sb[:], g_ps[:])
                            nc.gpsimd.dma_start(g_out[:], g_sb[:])

                # flush helper: copy a finished bank (alternating scalar/
                # vector — different PSUM banks run in parallel) and DMA it
                # out. bh0 outputs ride gpsimd (overlapped by bh1 compute);
                # bh1 outputs ride sync (idle by then).
                def flush(oc, name, last=False):
                    # bh0: scalar only, keeping the vector queue free to
                    # pre-build bh1's vT tiles. bh1 (kernel tail): alternate
                    # scalar/vector copies (different PSUM banks -> parallel)
                    # and sync/gpsimd DMA queues to parallelize the tail.
                    ust = ustp.tile([128, 512], F32, tag="ust", name=name)
                    if bh == 1 and oc % 2 == 1:
                        nc.vector.tensor_copy(ust[:], acc[oc][:])
                    else:
                        nc.scalar.copy(ust[:], acc[oc][:])
                    eng = nc.gpsimd if bh == 0 else nc.sync
                    eng.dma_start(u_out_r[oc][:, bsl], ust[:])

                def build_vt8(p, name):
                    vt8 = vt8p.tile([128, 2, 512], F8, tag="vt8", name=name)
                    for m in (0, 1):
                        tl = NT16 + 2 * p + m            # local k-tile index
                        il8 = tl // 3 - NI16              # index into hsb8
                        jb = tl % 3
                        nc.vector.tensor_mul(
                            vt8[:, m, :], htt_slice(bh, jb), hsb8_t[:, il8, bsl]
                        )
                    return vt8

                # fp8 DoubleRow pairs. The last weight batch runs oc-major
                # (all its pairs for one output chunk, then the next) so each
                # bank finishes, flushes, and DMAs out ~8us before the last
                # matmul — the tail exposes only one 256KB DMA, and the next
                # bh's banks are already free when its stream starts.
                if NP8:
                    p = 0
                    for pi, _ in enumerate(p_batches):
                        w8_t, p0, npb = w2_tiles[("f8", bh, pi)]
                        if pi < len(p_batches) - 1:
                            for pp in range(npb):
                                vt8 = build_vt8(p, f"vt8_{bh}_{p}")
                                for oc in range(6):
                                    nc.tensor.matmul(
                                        acc[oc][:],
                                        w8_t[:, pp, :, oc * 128 : (oc + 1) * 128],
                                        vt8[:],
                                        perf_mode=DR,
                                        start=False,
                                        stop=False,
                                    )
                                p += 1
                        else:
                            vt8s = [
                                build_vt8(p + pp, f"vt8e_{bh}_{pp}")
                                for pp in range(npb)
                            ]
                            for oc in range(6):
                                for pp in range(npb):
                                    nc.tensor.matmul(
                                        acc[oc][:],
                                        w8_t[:, pp, :, oc * 128 : (oc + 1) * 128],
                                        vt8s[pp][:],
                                        perf_mode=DR,
                                        start=False,
                                        stop=(pp == npb - 1),
                                    )
                                flush(oc, f"uste_{bh}_{oc}", last=(oc == 5))
                            p += npb
                else:
                    for oc in range(6):
                        flush(oc, f"ust_{bh}_{oc}")

    _split_excess_waits(nc)
    _strip_unused_mm_incs(nc)
    return nc


_NC_CACHE = None


def kernel(hspatial, htext, W_L, b_L, W_L2, b_L2):
    global LAST_EXEC_TIME_NS, LAST_RESULTS, _NC_CACHE

    hs = np.asarray(hspatial, dtype=np.float32)
    ht = np.asarray(htext, dtype=np.float32)
    W_L = np.asarray(W_L, dtype=np.float32)
    b_L = np.asarray(b_L, dtype=np.float32)
    W_L2 = np.asarray(W_L2, dtype=np.float32)
    b_L2 = np.asarray(b_L2, dtype=np.float32)

    htt = np.ascontiguousarray(ht.T.astype(NP_F16))        # [D, B]
    x = np.concatenate([hs, ht], axis=1)                   # [B, 2D]
    xt = np.ascontiguousarray(x.T.astype(NP_F16))          # [2D, B]
    wlt = np.ascontiguousarray(W_L.T.astype(NP_F16))       # [2D, 2D]

    in_maps = []
    for c in range(NCORES):
        sh0 = c * KC
        w2t_c = np.ascontiguousarray(
            W_L2[:, sh0 : sh0 + NT16 * 128].T.astype(NP_F16)
        )                                                   # [NT16*128, 2D]
        hs_c = hs[:, c * IC : c * IC + NI16].T.astype(NP_F16)   # [NI16, B]
        hsb16_c = np.ascontiguousarray(
            np.broadcast_to(hs_c[:, None, :], (max(NI16, 1), 128, B))
        )
        xt_c = np.ascontiguousarray(xt[:, c * 128 : (c + 1) * 128])
        m = {
            "w2t": w2t_c,
            "hsb16": hsb16_c,
            "htt": htt,
            "xt": xt_c,
            "wlt": wlt,
        }
        if NP8:
            W8 = (
                W_L2[:, sh0 + NT16 * 128 : sh0 + KC] * np.float32(W8SCALE)
            ).T.astype(NP_F8)                               # [NI8*D, 2D]
            w28_c = np.ascontiguousarray(
                W8.reshape(NP8, 2, 128, TWO_D)
                .transpose(0, 2, 1, 3)
                .reshape(NP8, 128, 2 * TWO_D)
            )
            hs8_c = (
                hs[:, c * IC + NI16 : (c + 1) * IC].T / np.float32(W8SCALE)
            ).astype(NP_F16)                                # [NI8, B]
            hsb8_c = np.ascontiguousarray(
                np.broadcast_to(hs8_c[:, None, :], (NI8, 128, B))
            )
            m["w28"] = w28_c
            m["hsb8"] = hsb8_c
        in_maps.append(m)

    if _NC_CACHE is None:
        _NC_CACHE = _gen()
    nc = _NC_CACHE

    res = run_bass_kernel_spmd(nc, in_maps, core_ids=list(range(NCORES)))
    LAST_EXEC_TIME_NS = res.exec_time_ns
    LAST_RESULTS = res

    ut = np.zeros((TWO_D, B), dtype=np.float64)
    for c in range(NCORES):
        ut += res.results[c]["u_out"]
    u = (ut.T + b_L2).astype(np.float32)

    g = np.concatenate([res.results[c]["g_out"] for c in range(NCORES)], axis=0)
    g = g + b_L

    return np.concatenate([g, u], axis=1).astype(np.float32)


# revision 6
# speedup vs baseline: 1.0308x; 1.0308x over previous
"""DeepAndWide Trainium2 kernel (8 NeuronCores, SPMD via Bass/Tile).

Math:
    g = concat(hs, ht, 1) @ W_L.T + b_L            # [B, 2D] deep path
    v = outer(hs_b, ht_b) flattened -> [B, D*D]    # wide-path features
    u = v @ W_L2.T + b_L2                          # [B, 2D]
    out = concat(g, u, 1)                          # [B, 4D]

Sharding: W_L2 column-sharded over the D*D contraction dim across 8 cores
(each core owns 48 outer-product rows i); every core computes a partial u
over the full batch; the host sums the partials. g is data-parallel over
batch (128 rows per core).

v2 structure (vs v1):
  * Full-contraction PSUM accumulation: 6 PSUM banks hold uT[oc] for one
    batch half (bh) across the whole local contraction; no SBUF fp32
    accumulator, no per-group DVE adds. Weights are re-streamed per bh
    (2x weight DMA, still well under the PE-bound span).
  * fp8 DoubleRow for the last NI8 of 48 i's: W slice pre-scaled x16 and
    quantized e4m3 on host; hs slice pre-scaled 1/16 so the DVE-built
    vT tiles land in e4m3 range. One DoubleRow matmul contracts 2 k-tiles
    per 512-col stream (~1.4-1.8x the fp16 rate). Rel-err budget measured
    offline: alpha=0.25 -> ~1.6e-2 (< 2e-2 gate).
  * Dedicated engine queues (no head-of-line blocking): sync=weights,
    scalar=htt/xt/wlt DMA + all PSUM->SBUF flush copies, gpsimd=hsb +
    bh0/g outputs, vector=vT builds only, tensor=matmuls.
  * Staggered per-oc flush at each bh end; bh0 outputs DMA during bh1
    compute; deep path g slotted mid-bh0.
"""

import os as _os

import numpy as np

import concourse.bass as bass
import concourse.mybir as mybir
from concourse import tile
from concourse.bass_utils import run_bass_kernel_spmd

B = 1024
D = 384
NCORES = 8
IC = D // NCORES          # 48 outer-product rows (i) per core
KC = IC * D               # 18432 contraction columns per core
TWO_D = 2 * D             # 768

NI8 = int(_os.environ.get("KERNEL_NI8", "14"))   # fp8 i's per core (even)
assert NI8 % 2 == 0 and 0 <= NI8 <= IC
NI16 = IC - NI8           # fp16 i's per core
NT16 = NI16 * 3           # fp16 k-tiles (128 rows each)
NP8 = (NI8 * 3) // 2      # fp8 DoubleRow pairs (2 k-tiles each)
W8SCALE = 16.0            # host: W8 = W*16, hs8 = hs/16 (product scale = 1)

WB = 6                    # fp16 k-tiles per weight DMA batch
PB = 6                    # fp8 pairs per weight DMA batch

F32 = mybir.dt.float32
F16 = mybir.dt.float16
F8 = mybir.dt.float8e4
NP_F16 = np.float16
NP_F8 = mybir.dt.np(F8)
DR = mybir.MatmulPerfMode.DoubleRow

LAST_EXEC_TIME_NS = None
LAST_RESULTS = None


def _split_excess_waits(nc):
    """walrus rejects >1 sync-wait on several instruction structs (fp32/f32r
    Matmult, Drain, ...). Hoist all but the last wait of any multi-wait
    instruction onto single-wait EventSemaphore instructions inserted just
    before it on the same engine."""
    n = [0]

    def fresh():
        n[0] += 1
        return f"WSPLIT-{n[0]}"

    for f in nc.m.functions:
        for blk in f.blocks:
            out = []
            changed = False
            for ins in blk.instructions:
                si = ins.sync_info
                if si is not None and len(si.on_wait) > 1:
                    waits = list(si.on_wait)
                    for w in waits[:-1]:
                        ev = mybir.InstEventSemaphore(
                            name=fresh(),
                            engine=ins.engine,
                            ins=[],
                            outs=[],
                            sync_info=mybir.SyncInfo(on_wait=[w], on_update=[]),
                        )
                        out.append(ev)
                    ins.sync_info = mybir.SyncInfo(
                        on_wait=[waits[-1]], on_update=list(si.on_update)
                    )
                    changed = True
                out.append(ins)
            if changed:
                blk.instructions = out


def _strip_unused_mm_incs(nc):
    """Every matmul carries a +1 update on the PE semaphore; the EVT_SEM
    register write costs the PE queue ~26ns each. Keep only the increments
    whose cumulative tick some wait actually references (plus the final
    one), and renumber all waits on that semaphore accordingly."""
    from collections import defaultdict

    for f in nc.m.functions:
        upd_insts = defaultdict(list)
        wait_refs = defaultdict(list)
        for blk in f.blocks:
            for ins in blk.instructions:
                si = ins.sync_info
                if not si:
                    continue
                for u in si.on_update:
                    upd_insts[u.id].append((ins, u))
                for w in si.on_wait:
                    wait_refs[w.id].append(w)

        for sem_id, upds in upd_insts.items():
            if not all(
                type(i).__name__ == "InstMatmult"
                and u.update_mode == "sem-inc"
                and u.update_value == 1
                for i, u in upds
            ):
                continue
            ws = wait_refs.get(sem_id, [])
            if any(
                w.wait_mode != "sem-ge-imm" or w.wait_reg is not None for w in ws
            ):
                continue
            used = {w.wait_value for w in ws}
            n = len(upds)
            keep = []
            kept_prefix = []
            kept = 0
            for tick in range(1, n + 1):
                k = tick in used or tick == n
                keep.append(k)
                kept += 1 if k else 0
                kept_prefix.append(kept)
            for (ins, u), k in zip(upds, keep):
                if not k:
                    si = ins.sync_info
                    ins.sync_info = mybir.SyncInfo(
                        on_wait=list(si.on_wait),
                        on_update=[x for x in si.on_update if x.id != sem_id],
                    )
            for w in ws:
                v = w.wait_value
                if v >= 1:
                    w.wait_value = kept_prefix[min(v, n) - 1]


def _gen():
    nc = bass.Bass()

    w2t = nc.dram_tensor("w2t", [NT16 * 128, TWO_D], F16, kind="ExternalInput")
    hsb16 = nc.dram_tensor("hsb16", [max(NI16, 1), 128, B], F16, kind="ExternalInput")
    htt = nc.dram_tensor("htt", [D, B], F16, kind="ExternalInput")
    xt = nc.dram_tensor("xt", [TWO_D, 128], F16, kind="ExternalInput")
    wlt = nc.dram_tensor("wlt", [TWO_D, TWO_D], F16, kind="ExternalInput")
    if NP8:
        w28 = nc.dram_tensor("w28", [NP8, 128, 2 * TWO_D], F8, kind="ExternalInput")
        hsb8 = nc.dram_tensor("hsb8", [NI8, 128, B], F16, kind="ExternalInput")
    u_out = nc.dram_tensor("u_out", [TWO_D, B], F32, kind="ExternalOutput")  # uT
    g_out = nc.dram_tensor("g_out", [128, TWO_D], F32, kind="ExternalOutput")

    u_out_r = u_out.rearrange("(c p) b -> c p b", p=128)          # [6,128,B]
    w2_pto = w2t.rearrange("(t p) o -> p t o", p=128)             # [128,NT16,2D]
    hsb16_pib = hsb16.rearrange("i p b -> p i b")                 # [128,NI16,B]
    htt_pjb = htt.rearrange("(j p) b -> p j b", p=128)            # [128,3,B]
    if NP8:
        w28_pnq = w28.rearrange("n p q -> p n q")                 # [128,NP8,1536]
        hsb8_pib = hsb8.rearrange("i p b -> p i b")               # [128,NI8,B]

    # fp16 weight batching: first 3 fine allocations (1,2,3 tiles) for a fast
    # ramp, then WB-tile batches. Same batch schedule reused for bh1 except
    # everything coarse.
    fine = [1, 2, 3, 3, 3]
    rest = NT16 - sum(fine)
    assert rest % WB == 0
    batches_bh0 = fine + [WB] * (rest // WB)
    batches_bh1 = [WB] * (NT16 // WB)
    p_batches = [PB] * (NP8 // PB) if NP8 else []
    if NP8 % PB:
        p_batches.append(NP8 % PB)

    with tile.TileContext(nc) as tc:
        with (
            tc.tile_pool(name="const", bufs=1) as constp,
            tc.tile_pool(name="w2p", bufs=3) as w2p,
            tc.tile_pool(name="w8p", bufs=2) as w8p,
            tc.tile_pool(name="vtp", bufs=8) as vtp,
            tc.tile_pool(name="vt8p", bufs=7) as vt8p,
            tc.tile_pool(name="accp", bufs=1, space="PSUM") as accp,
            tc.tile_pool(name="gpsum", bufs=1, space="PSUM") as gps,
            tc.tile_pool(name="ustp", bufs=8) as ustp,
            tc.tile_pool(name="gpool", bufs=1) as gp,
        ):
            # ---- constants / inputs resident in SBUF ----
            # bh0's htt split so the first vT build waits only on a 128KB DMA
            htt_a = constp.tile([128, 512], F16, tag="htta")       # bh0 j0
            htt_bc = constp.tile([128, 2, 512], F16, tag="httbc")  # bh0 j1,2
            htt_d = constp.tile([128, 3, 512], F16, tag="httd")    # bh1 all j

            def htt_slice(bh, jb):
                if bh == 1:
                    return htt_d[:, jb, :]
                return htt_a[:] if jb == 0 else htt_bc[:, jb - 1, :]
            # hsb fp16: 4 single-i tiles (fine deps early), then batches of 4;
            # i0 additionally split by batch half so the very first vT build
            # waits on a 128KB DMA only.
            hsb_s0h = [
                constp.tile([128, 512], F16, tag=f"hsbs0h{h}", name=f"hsbs0h{h}")
                for h in (0, 1)
            ]
            hsb_s = [
                constp.tile([128, B], F16, tag=f"hsbs{i}", name=f"hsbs{i}")
                for i in range(1, 4)
            ]
            hb_sizes = []
            left = NI16 - 4
            while left > 0:
                hb_sizes.append(min(4, left))
                left -= 4
            n_hb = len(hb_sizes)
            hsb_b = [
                constp.tile([128, hb_sizes[n], B], F16, tag=f"hsbb{n}",
                            name=f"hsbb{n}")
                for n in range(n_hb)
            ]
            if NP8:
                hsb8_t = constp.tile([128, NI8, B], F16, tag="hsb8")
            wlt_t = gp.tile([128, 6, TWO_D], F16, tag="wlt")
            xt_t = gp.tile([128, 6, 128], F16, tag="xtt")
            g_sb = gp.tile([128, TWO_D], F32, tag="gsb")
            g_ps = gps.tile([128, TWO_D], F32, tag="gps")

            def hsb_slice(il, bh):
                bsl = slice(bh * 512, (bh + 1) * 512)
                if il == 0:
                    return hsb_s0h[bh][:]
                if il < 4:
                    return hsb_s[il - 1][:, bsl]
                n, r = divmod(il - 4, 4)
                return hsb_b[n][:, r, bsl]

            # ---- critical-path DMA issue (dedicated queues) ----
            # scalar: first-tile deps (htt j0, hs i0) then the rest of htt,
            # hs i1, and g inputs; scalar also runs the flush copies later.
            nc.sync.dma_start(hsb_s0h[0][:], hsb16[0][:, 0:512])
            nc.scalar.dma_start(htt_a[:], htt_pjb[:, 0, 0:512])
            nc.scalar.dma_start(htt_bc[:], htt_pjb[:, 1:3, 0:512])
            nc.scalar.dma_start(hsb_s0h[1][:], hsb16[0][:, 512:1024])
            nc.scalar.dma_start(hsb_s[0][:], hsb16[1])
            nc.scalar.dma_start(xt_t[:], xt.rearrange("(t p) c -> p t c", p=128))
            # gpsimd: hs i2, i3, first hs batch (needed from t=12)
            nc.gpsimd.dma_start(hsb_s[1][:], hsb16[2])
            nc.gpsimd.dma_start(hsb_s[2][:], hsb16[3])
            nc.gpsimd.dma_start(
                hsb_b[0][:], hsb16_pib[:, 4 : 4 + hb_sizes[0], :]
            )

            # sync: all wide-path weights, ring-throttled prefetch. The
            # remaining hs broadcast batches + hsb8 are interleaved into this
            # stream so they inherit its pacing instead of flooding the DMA
            # fabric up-front (which starves the PE of weights early on).
            w2_tiles = {}   # (bh, batch_idx) -> (tile, t0, nt)
            for bh in (0, 1):
                t0 = 0
                nb6 = 0
                for bi, nt in enumerate(batches_bh0 if bh == 0 else batches_bh1):
                    w2_t = w2p.tile([128, nt, TWO_D], F16, tag="w2")
                    nc.sync.dma_start(w2_t[:], w2_pto[:, t0 : t0 + nt, :])
                    w2_tiles[(bh, bi)] = (w2_t, t0, nt)
                    t0 += nt
                    if bh == 0 and nt == WB:
                        nb6 += 1
                        if nb6 == 4:
                            nc.sync.dma_start(
                                wlt_t[:],
                                wlt.rearrange("(t p) o -> p t o", p=128),
                            )
                        if nb6 == 6:
                            nc.sync.dma_start(
                                htt_d[:], htt_pjb[:, :, 512:1024]
                            )
                        if nb6 % 2 == 1 and (nb6 + 1) // 2 < n_hb:
                            n = (nb6 + 1) // 2
                            nc.sync.dma_start(
                                hsb_b[n][:],
                                hsb16_pib[:, 4 + 4 * n : 4 + 4 * n + hb_sizes[n], :],
                            )
                        if NP8 and nb6 == 10:
                            nc.sync.dma_start(hsb8_t[:], hsb8_pib[:])
                if NP8:
                    p0 = 0
                    for pi, npb in enumerate(p_batches):
                        w8_t = w8p.tile([128, npb, 2, TWO_D], F8, tag="w8")
                        nc.sync.dma_start(
                            w8_t.rearrange("p n two o -> p n (two o)"),
                            w28_pnq[:, p0 : p0 + npb, :],
                        )
                        w2_tiles[("f8", bh, pi)] = (w8_t, p0, npb)
                        p0 += npb

            # ---- main loops ----
            NACC = NT16 + (1 if NP8 else 0)  # accumulation "steps" per bank
            for bh in (0, 1):
                bsl = slice(bh * 512, (bh + 1) * 512)
                acc = [
                    accp.tile([128, 512], F32, tag=f"acc{oc}", name=f"acc{oc}")
                    for oc in range(6)
                ]

                # fp16 k-tiles
                t = 0
                for bi, _ in enumerate(batches_bh0 if bh == 0 else batches_bh1):
                    w2_t, t0, nt = w2_tiles[(bh, bi)]
                    for tt in range(nt):
                        il, jb = divmod(t, 3)
                        vt = vtp.tile([128, 512], F16, tag="vt")
                        nc.vector.tensor_mul(
                            vt[:], htt_slice(bh, jb), hsb_slice(il, bh)
                        )
                        for oc in range(6):
                            nc.tensor.matmul(
                                acc[oc][:],
                                w2_t[:, tt, oc * 128 : (oc + 1) * 128],
                                vt[:],
                                start=(t == 0),
                                stop=(t == NT16 - 1 and not NP8),
                            )
                        t += 1

                        # deep path: slotted mid-bh0 (inputs arrived long ago;
                        # PE warm; keeps both kernel head and tail clear)
                        if bh == 0 and t == 36:
                            for gt in range(6):
                                nc.tensor.matmul(
                                    g_ps[:, 0:512],
                                    xt_t[:, gt, :],
                                    wlt_t[:, gt, 0:512],
                                    start=(gt == 0),
                                    stop=(gt == 5),
                                )
                                nc.tensor.matmul(
                                    g_ps[:, 512:768],
                                    xt_t[:, gt, :],
                                    wlt_t[:, gt, 512:768],
                                    start=(gt == 0),
                                    stop=(gt == 5),
                                )
                            nc.scalar.copy(g_sb[:], g_ps[:])
                            nc.gpsimd.dma_start(g_out[:], g_sb[:])

                # flush helper: copy a finished bank (alternating scalar/
                # vector — different PSUM banks run in parallel) and DMA it
                # out. bh0 outputs ride gpsimd (overlapped by bh1 compute);
                # bh1 outputs ride sync (idle by then).
                def flush(oc, name, last=False):
                    # bh0: scalar only, keeping the vector queue free to
                    # pre-build bh1's vT tiles. bh1 (kernel tail): alternate
                    # scalar/vector copies (different PSUM banks -> parallel)
                    # and sync/gpsimd DMA queues to parallelize the tail.
                    ust = ustp.tile([128, 512], F32, tag="ust", name=name)
                    if bh == 1 and oc % 2 == 1:
                        nc.vector.tensor_copy(ust[:], acc[oc][:])
                    else:
                        nc.scalar.copy(ust[:], acc[oc][:])
                    eng = nc.gpsimd if bh == 0 else nc.sync
                    eng.dma_start(u_out_r[oc][:, bsl], ust[:])

                def build_vt8(p, name):
                    vt8 = vt8p.tile([128, 2, 512], F8, tag="vt8", name=name)
                    for m in (0, 1):
                        tl = NT16 + 2 * p + m            # local k-tile index
                        il8 = tl // 3 - NI16              # index into hsb8
                        jb = tl % 3
                        nc.vector.tensor_mul(
                            vt8[:, m, :], htt_slice(bh, jb), hsb8_t[:, il8, bsl]
                        )
                    return vt8

                # fp8 DoubleRow pairs. The last weight batch runs oc-major
                # (all its pairs for one output chunk, then the next) so each
                # bank finishes, flushes, and DMAs out ~8us before the last
                # matmul — the tail exposes only one 256KB DMA, and the next
                # bh's banks are already free when its stream starts.
                if NP8:
                    p = 0
                    for pi, _ in enumerate(p_batches):
                        w8_t, p0, npb = w2_tiles[("f8", bh, pi)]
                        if pi < len(p_batches) - 1:
                            for pp in range(npb):
                                vt8 = build_vt8(p, f"vt8_{bh}_{p}")
                                for oc in range(6):
                                    nc.tensor.matmul(
                                        acc[oc][:],
                                        w8_t[:, pp, :, oc * 128 : (oc + 1) * 128],
                                        vt8[:],
                                        perf_mode=DR,
                                        start=False,
                                        stop=False,
                                    )
                                p += 1
                        else:
                            vt8s = [
                                build_vt8(p + pp, f"vt8e_{bh}_{pp}")
                                for pp in range(npb)
                            ]
                            for oc in range(6):
                                for pp in range(npb):
                                    nc.tensor.matmul(
                                        acc[oc][:],
                                        w8_t[:, pp, :, oc * 128 : (oc + 1) * 128],
                                        vt8s[pp][:],
                                        perf_mode=DR,
                                        start=False,
                                        stop=(pp == npb - 1),
                                    )
                                flush(oc, f"uste_{bh}_{oc}", last=(oc == 5))
                            p += npb
                else:
                    for oc in range(6):
                        flush(oc, f"ust_{bh}_{oc}")

    _split_excess_waits(nc)
    _strip_unused_mm_incs(nc)
    return nc


_NC_CACHE = None


def kernel(hspatial, htext, W_L, b_L, W_L2, b_L2):
    global LAST_EXEC_TIME_NS, LAST_RESULTS, _NC_CACHE

    hs = np.asarray(hspatial, dtype=np.float32)
    ht = np.asarray(htext, dtype=np.float32)
    W_L = np.asarray(W_L, dtype=np.float32)
    b_L = np.asarray(b_L, dtype=np.float32)
    W_L2 = np.asarray(W_L2, dtype=np.float32)
    b_L2 = np.asarray(b_L2, dtype=np.float32)

    htt = np.ascontiguousarray(ht.T.astype(NP_F16))        # [D, B]
    x = np.concatenate([hs, ht], axis=1)                   # [B, 2D]
    xt = np.ascontiguousarray(x.T.astype(NP_F16))          # [2D, B]
    wlt = np.ascontiguousarray(W_L.T.astype(NP_F16))       # [2D, 2D]

    in_maps = []
    for c in range(NCORES):
        sh0 = c * KC
        w2t_c = np.ascontiguousarray(
            W_L2[:, sh0 : sh0 + NT16 * 128].T.astype(NP_F16)
        )                                                   # [NT16*128, 2D]
        hs_c = hs[:, c * IC : c * IC + NI16].T.astype(NP_F16)   # [NI16, B]
        hsb16_c = np.ascontiguousarray(
            np.broadcast_to(hs_c[:, None, :], (max(NI16, 1), 128, B))
        )
        xt_c = np.ascontiguousarray(xt[:, c * 128 : (c + 1) * 128])
        m = {
            "w2t": w2t_c,
            "hsb16": hsb16_c,
            "htt": htt,
            "xt": xt_c,
            "wlt": wlt,
        }
        if NP8:
            W8 = (
                W_L2[:, sh0 + NT16 * 128 : sh0 + KC] * np.float32(W8SCALE)
            ).T.astype(NP_F8)                               # [NI8*D, 2D]
            w28_c = np.ascontiguousarray(
                W8.reshape(NP8, 2, 128, TWO_D)
                .transpose(0, 2, 1, 3)
                .reshape(NP8, 128, 2 * TWO_D)
            )
            hs8_c = (
                hs[:, c * IC + NI16 : (c + 1) * IC].T / np.float32(W8SCALE)
            ).astype(NP_F16)                                # [NI8, B]
            hsb8_c = np.ascontiguousarray(
                np.broadcast_to(hs8_c[:, None, :], (NI8, 128, B))
            )
            m["w28"] = w28_c
            m["hsb8"] = hsb8_c
        in_maps.append(m)

    if _NC_CACHE is None:
        _NC_CACHE = _gen()
    nc = _NC_CACHE

    res = run_bass_kernel_spmd(nc, in_maps, core_ids=list(range(NCORES)))
    LAST_EXEC_TIME_NS = res.exec_time_ns
    LAST_RESULTS = res

    ut = np.zeros((TWO_D, B), dtype=np.float64)
    for c in range(NCORES):
        ut += res.results[c]["u_out"]
    u = (ut.T + b_L2).astype(np.float32)

    g = np.concatenate([res.results[c]["g_out"] for c in range(NCORES)], axis=0)
    g = g + b_L

    return np.concatenate([g, u], axis=1).astype(np.float32)


# revision 8
# speedup vs baseline: 1.0543x; 1.0228x over previous
"""DeepAndWide Trainium2 kernel (8 NeuronCores, SPMD via Bass/Tile).

Math:
    g = concat(hs, ht, 1) @ W_L.T + b_L            # [B, 2D] deep path
    v = outer(hs_b, ht_b) flattened -> [B, D*D]    # wide-path features
    u = v @ W_L2.T + b_L2                          # [B, 2D]
    out = concat(g, u, 1)                          # [B, 4D]

Sharding: W_L2 column-sharded over the D*D contraction dim across 8 cores
(each core owns 48 outer-product rows i); every core computes a partial u
over the full batch; the host sums the partials. g is data-parallel over
batch (128 rows per core).

v2 structure (vs v1):
  * Full-contraction PSUM accumulation: 6 PSUM banks hold uT[oc] for one
    batch half (bh) across the whole local contraction; no SBUF fp32
    accumulator, no per-group DVE adds. Weights are re-streamed per bh
    (2x weight DMA, still well under the PE-bound span).
  * fp8 DoubleRow for the last NI8 of 48 i's: W slice pre-scaled x16 and
    quantized e4m3 on host; hs slice pre-scaled 1/16 so the DVE-built
    vT tiles land in e4m3 range. One DoubleRow matmul contracts 2 k-tiles
    per 512-col stream (~1.4-1.8x the fp16 rate). Rel-err budget measured
    offline: alpha=0.25 -> ~1.6e-2 (< 2e-2 gate).
  * Dedicated engine queues (no head-of-line blocking): sync=weights,
    scalar=htt/xt/wlt DMA + all PSUM->SBUF flush copies, gpsimd=hsb +
    bh0/g outputs, vector=vT builds only, tensor=matmuls.
  * Staggered per-oc flush at each bh end; bh0 outputs DMA during bh1
    compute; deep path g slotted mid-bh0.
"""

import os as _os

import numpy as np

import concourse.bass as bass
import concourse.mybir as mybir
from concourse import tile
from concourse.bass_utils import run_bass_kernel_spmd

B = 1024
D = 384
NCORES = 8
IC = D // NCORES          # 48 outer-product rows (i) per core
KC = IC * D               # 18432 contraction columns per core
TWO_D = 2 * D             # 768

NI8 = int(_os.environ.get("KERNEL_NI8", "16"))   # fp8 i's per core (even)
assert NI8 % 2 == 0 and 0 <= NI8 <= IC
NI16 = IC - NI8           # fp16 i's per core
NT16 = NI16 * 3           # fp16 k-tiles (128 rows each)
NP8 = (NI8 * 3) // 2      # fp8 DoubleRow pairs (2 k-tiles each)
W8SCALE = 16.0            # host: W8 = W*16, hs8 = hs/16 (product scale = 1)

WB = 6                    # fp16 k-tiles per weight DMA batch
PB = 6                    # fp8 pairs per weight DMA batch

F32 = mybir.dt.float32
F16 = mybir.dt.float16
F8 = mybir.dt.float8e4
NP_F16 = np.float16
NP_F8 = mybir.dt.np(F8)
DR = mybir.MatmulPerfMode.DoubleRow

LAST_EXEC_TIME_NS = None
LAST_RESULTS = None


def _split_excess_waits(nc):
    """walrus rejects >1 sync-wait on several instruction structs (fp32/f32r
    Matmult, Drain, ...). Hoist all but the last wait of any multi-wait
    instruction onto single-wait EventSemaphore instructions inserted just
    before it on the same engine."""
    n = [0]

    def fresh():
        n[0] += 1
        return f"WSPLIT-{n[0]}"

    for f in nc.m.functions:
        for blk in f.blocks:
            out = []
            changed = False
            for ins in blk.instructions:
                si = ins.sync_info
                if si is not None and len(si.on_wait) > 1:
                    waits = list(si.on_wait)
                    for w in waits[:-1]:
                        ev = mybir.InstEventSemaphore(
                            name=fresh(),
                            engine=ins.engine,
                            ins=[],
                            outs=[],
                            sync_info=mybir.SyncInfo(on_wait=[w], on_update=[]),
                        )
                        out.append(ev)
                    ins.sync_info = mybir.SyncInfo(
                        on_wait=[waits[-1]], on_update=list(si.on_update)
                    )
                    changed = True
                out.append(ins)
            if changed:
                blk.instructions = out


def _strip_unused_mm_incs(nc):
    """Every matmul carries a +1 update on the PE semaphore; the EVT_SEM
    register write costs the PE queue ~26ns each. Keep only the increments
    whose cumulative tick some wait actually references (plus the final
    one), and renumber all waits on that semaphore accordingly."""
    from collections import defaultdict

    for f in nc.m.functions:
        upd_insts = defaultdict(list)
        wait_refs = defaultdict(list)
        for blk in f.blocks:
            for ins in blk.instructions:
                si = ins.sync_info
                if not si:
                    continue
                for u in si.on_update:
                    upd_insts[u.id].append((ins, u))
                for w in si.on_wait:
                    wait_refs[w.id].append(w)

        for sem_id, upds in upd_insts.items():
            if not all(
                type(i).__name__ == "InstMatmult"
                and u.update_mode == "sem-inc"
                and u.update_value == 1
                for i, u in upds
            ):
                continue
            ws = wait_refs.get(sem_id, [])
            if any(
                w.wait_mode != "sem-ge-imm" or w.wait_reg is not None for w in ws
            ):
                continue
            used = {w.wait_value for w in ws}
            n = len(upds)
            keep = []
            kept_prefix = []
            kept = 0
            for tick in range(1, n + 1):
                k = tick in used or tick == n
                keep.append(k)
                kept += 1 if k else 0
                kept_prefix.append(kept)
            for (ins, u), k in zip(upds, keep):
                if not k:
                    si = ins.sync_info
                    ins.sync_info = mybir.SyncInfo(
                        on_wait=list(si.on_wait),
                        on_update=[x for x in si.on_update if x.id != sem_id],
                    )
            for w in ws:
                v = w.wait_value
                if v >= 1:
                    w.wait_value = kept_prefix[min(v, n) - 1]


def _gen():
    nc = bass.Bass()

    w2t = nc.dram_tensor("w2t", [NT16 * 128, TWO_D], F16, kind="ExternalInput")
    hsb16 = nc.dram_tensor("hsb16", [max(NI16, 1), 128, B], F16, kind="ExternalInput")
    htt = nc.dram_tensor("htt", [D, B], F16, kind="ExternalInput")
    xt = nc.dram_tensor("xt", [TWO_D, 128], F16, kind="ExternalInput")
    wlt = nc.dram_tensor("wlt", [TWO_D, TWO_D], F16, kind="ExternalInput")
    if NP8:
        w28 = nc.dram_tensor("w28", [NP8, 128, 2 * TWO_D], F8, kind="ExternalInput")
        hsb8 = nc.dram_tensor("hsb8", [NI8, 128, B], F16, kind="ExternalInput")
    u_out = nc.dram_tensor("u_out", [TWO_D, B], F32, kind="ExternalOutput")  # uT
    g_out = nc.dram_tensor("g_out", [128, TWO_D], F32, kind="ExternalOutput")

    u_out_r = u_out.rearrange("(c p) b -> c p b", p=128)          # [6,128,B]
    w2_pto = w2t.rearrange("(t p) o -> p t o", p=128)             # [128,NT16,2D]
    hsb16_pib = hsb16.rearrange("i p b -> p i b")                 # [128,NI16,B]
    htt_pjb = htt.rearrange("(j p) b -> p j b", p=128)            # [128,3,B]
    if NP8:
        w28_pnq = w28.rearrange("n p q -> p n q")                 # [128,NP8,1536]
        hsb8_pib = hsb8.rearrange("i p b -> p i b")               # [128,NI8,B]

    # fp16 weight batching: first 3 fine allocations (1,2,3 tiles) for a fast
    # ramp, then WB-tile batches. Same batch schedule reused for bh1 except
    # everything coarse.
    fine = [1, 2, 3, 3, 3]
    rest = NT16 - sum(fine)
    assert rest % WB == 0
    batches_bh0 = fine + [WB] * (rest // WB)
    batches_bh1 = [WB] * (NT16 // WB)
    p_batches = [PB] * (NP8 // PB) if NP8 else []
    if NP8 % PB:
        p_batches.append(NP8 % PB)

    with tile.TileContext(nc) as tc:
        with (
            tc.tile_pool(name="const", bufs=1) as constp,
            tc.tile_pool(name="w2p", bufs=3) as w2p,
            tc.tile_pool(name="w8p", bufs=2) as w8p,
            tc.tile_pool(name="vtp", bufs=8) as vtp,
            tc.tile_pool(name="vt8p", bufs=7) as vt8p,
            tc.tile_pool(name="accp", bufs=1, space="PSUM") as accp,
            tc.tile_pool(name="gpsum", bufs=1, space="PSUM") as gps,
            tc.tile_pool(name="ustp", bufs=8) as ustp,
            tc.tile_pool(name="gpool", bufs=1) as gp,
        ):
            # ---- constants / inputs resident in SBUF ----
            # bh0's htt split so the first vT build waits only on a 128KB DMA
            htt_a = constp.tile([128, 512], F16, tag="htta")       # bh0 j0
            htt_bc = constp.tile([128, 2, 512], F16, tag="httbc")  # bh0 j1,2
            htt_d = constp.tile([128, 3, 512], F16, tag="httd")    # bh1 all j

            def htt_slice(bh, jb):
                if bh == 1:
                    return htt_d[:, jb, :]
                return htt_a[:] if jb == 0 else htt_bc[:, jb - 1, :]
            # hsb fp16: 4 single-i tiles (fine deps early), then batches of 4;
            # i0 additionally split by batch half so the very first vT build
            # waits on a 128KB DMA only.
            hsb_s0h = [
                constp.tile([128, 512], F16, tag=f"hsbs0h{h}", name=f"hsbs0h{h}")
                for h in (0, 1)
            ]
            hsb_sh = {
                h: [
                    constp.tile([128, 512], F16, tag=f"hsbs{i}h{h}",
                                name=f"hsbs{i}h{h}")
                    for i in range(1, 4)
                ]
                for h in (0, 1)
            }
            hb_sizes = []
            left = NI16 - 4
            while left > 0:
                hb_sizes.append(min(4, left))
                left -= 4
            n_hb = len(hb_sizes)
            hsb_b0h = {
                h: constp.tile([128, hb_sizes[0], 512], F16, tag=f"hsbb0h{h}",
                               name=f"hsbb0h{h}")
                for h in (0, 1)
            }
            hsb_b = [None] + [
                constp.tile([128, hb_sizes[n], B], F16, tag=f"hsbb{n}",
                            name=f"hsbb{n}")
                for n in range(1, n_hb)
            ]
            if NP8:
                hsb8_t = constp.tile([128, NI8, B], F16, tag="hsb8")
            wlt_t = gp.tile([128, 6, TWO_D], F16, tag="wlt")
            xt_t = gp.tile([128, 6, 128], F16, tag="xtt")
            g_sb = gp.tile([128, TWO_D], F32, tag="gsb")
            g_ps = gps.tile([128, TWO_D], F32, tag="gps")

            def hsb_slice(il, bh):
                if il == 0:
                    return hsb_s0h[bh][:]
                if il < 4:
                    return hsb_sh[bh][il - 1][:]
                n, r = divmod(il - 4, 4)
                if n == 0:
                    return hsb_b0h[bh][:, r, :]
                return hsb_b[n][:, r, bh * 512 : (bh + 1) * 512]

            # ---- critical-path DMA issue (dedicated queues) ----
            # scalar: first-tile deps (htt j0, hs i0) then the rest of htt,
            # hs i1, and g inputs; scalar also runs the flush copies later.
            nc.sync.dma_start(hsb_s0h[0][:], hsb16[0][:, 0:512])
            nc.scalar.dma_start(htt_a[:], htt_pjb[:, 0, 0:512])
            nc.scalar.dma_start(htt_bc[:], htt_pjb[:, 1:3, 0:512])
            nc.scalar.dma_start(hsb_sh[0][0][:], hsb16[1][:, 0:512])
            # gpsimd: hs i2, i3 and batch-0 bh0 halves (needed from t=3/t=12);
            # all bh1 halves ride the paced sync stream much later.
            nc.gpsimd.dma_start(hsb_sh[0][1][:], hsb16[2][:, 0:512])
            nc.gpsimd.dma_start(hsb_sh[0][2][:], hsb16[3][:, 0:512])
            nc.gpsimd.dma_start(
                hsb_b0h[0][:], hsb16_pib[:, 4 : 4 + hb_sizes[0], 0:512]
            )

            # sync: all wide-path weights, ring-throttled prefetch. The
            # remaining hs broadcast batches + hsb8 are interleaved into this
            # stream so they inherit its pacing instead of flooding the DMA
            # fabric up-front (which starves the PE of weights early on).
            w2_tiles = {}   # (bh, batch_idx) -> (tile, t0, nt)
            for bh in (0, 1):
                t0 = 0
                nb6 = 0
                for bi, nt in enumerate(batches_bh0 if bh == 0 else batches_bh1):
                    w2_t = w2p.tile([128, nt, TWO_D], F16, tag="w2")
                    nc.sync.dma_start(w2_t[:], w2_pto[:, t0 : t0 + nt, :])
                    w2_tiles[(bh, bi)] = (w2_t, t0, nt)
                    t0 += nt
                    if bh == 0 and nt == WB:
                        nb6 += 1
                        if nb6 == 3:
                            nc.sync.dma_start(
                                xt_t[:],
                                xt.rearrange("(t p) c -> p t c", p=128),
                            )
                        if nb6 == 4:
                            nc.sync.dma_start(
                                wlt_t[:],
                                wlt.rearrange("(t p) o -> p t o", p=128),
                            )
                        if nb6 == 8:
                            nc.sync.dma_start(
                                hsb_s0h[1][:], hsb16[0][:, 512:1024]
                            )
                            nc.sync.dma_start(
                                hsb_sh[1][0][:], hsb16[1][:, 512:1024]
                            )
                        if nb6 == 9:
                            nc.sync.dma_start(
                                hsb_sh[1][1][:], hsb16[2][:, 512:1024]
                            )
                            nc.sync.dma_start(
                                hsb_sh[1][2][:], hsb16[3][:, 512:1024]
                            )
                        if nb6 == 11:
                            nc.sync.dma_start(
                                hsb_b0h[1][:],
                                hsb16_pib[:, 4 : 4 + hb_sizes[0], 512:1024],
                            )
                        if nb6 == 6:
                            nc.sync.dma_start(
                                htt_d[:], htt_pjb[:, :, 512:1024]
                            )
                        if nb6 % 2 == 1 and (nb6 + 1) // 2 < n_hb:
                            n = (nb6 + 1) // 2
                            nc.sync.dma_start(
                                hsb_b[n][:],
                                hsb16_pib[:, 4 + 4 * n : 4 + 4 * n + hb_sizes[n], :],
                            )
                        if NP8 and nb6 == 10:
                            nc.sync.dma_start(hsb8_t[:], hsb8_pib[:])
                if NP8:
                    p0 = 0
                    for pi, npb in enumerate(p_batches):
                        w8_t = w8p.tile([128, npb, 2, TWO_D], F8, tag="w8")
                        nc.sync.dma_start(
                            w8_t.rearrange("p n two o -> p n (two o)"),
                            w28_pnq[:, p0 : p0 + npb, :],
                        )
                        w2_tiles[("f8", bh, pi)] = (w8_t, p0, npb)
                        p0 += npb

            # ---- main loops ----
            NACC = NT16 + (1 if NP8 else 0)  # accumulation "steps" per bank
            for bh in (0, 1):
                bsl = slice(bh * 512, (bh + 1) * 512)
                acc = [
                    accp.tile([128, 512], F32, tag=f"acc{oc}", name=f"acc{oc}")
                    for oc in range(6)
                ]

                # fp16 k-tiles
                t = 0
                for bi, _ in enumerate(batches_bh0 if bh == 0 else batches_bh1):
                    w2_t, t0, nt = w2_tiles[(bh, bi)]
                    for tt in range(nt):
                        il, jb = divmod(t, 3)
                        vt = vtp.tile([128, 512], F16, tag="vt")
                        nc.vector.tensor_mul(
                            vt[:], htt_slice(bh, jb), hsb_slice(il, bh)
                        )
                        for oc in range(6):
                            nc.tensor.matmul(
                                acc[oc][:],
                                w2_t[:, tt, oc * 128 : (oc + 1) * 128],
                                vt[:],
                                start=(t == 0),
                                stop=(t == NT16 - 1 and not NP8),
                            )
                        t += 1

                        # deep path: slotted mid-bh0 (inputs arrived long ago;
                        # PE warm; keeps both kernel head and tail clear)
                        if bh == 0 and t == 36:
                            for gt in range(6):
                                nc.tensor.matmul(
                                    g_ps[:, 0:512],
                                    xt_t[:, gt, :],
                                    wlt_t[:, gt, 0:512],
                                    start=(gt == 0),
                                    stop=(gt == 5),
                                )
                                nc.tensor.matmul(
                                    g_ps[:, 512:768],
                                    xt_t[:, gt, :],
                                    wlt_t[:, gt, 512:768],
                                    start=(gt == 0),
                                    stop=(gt == 5),
                                )
                            nc.scalar.copy(g_sb[:], g_ps[:])
                            nc.gpsimd.dma_start(g_out[:], g_sb[:])

                # flush helper: copy a finished bank (alternating scalar/
                # vector — different PSUM banks run in parallel) and DMA it
                # out. bh0 outputs ride gpsimd (overlapped by bh1 compute);
                # bh1 outputs ride sync (idle by then).
                def flush(oc, name, last=False):
                    # bh0: scalar only, keeping the vector queue free to
                    # pre-build bh1's vT tiles. bh1 (kernel tail): alternate
                    # scalar/vector copies (different PSUM banks -> parallel)
                    # and sync/gpsimd DMA queues to parallelize the tail.
                    ust = ustp.tile([128, 512], F32, tag="ust", name=name)
                    if bh == 1 and oc % 2 == 1:
                        nc.vector.tensor_copy(ust[:], acc[oc][:])
                    else:
                        nc.scalar.copy(ust[:], acc[oc][:])
                    eng = nc.gpsimd if bh == 0 else nc.sync
                    eng.dma_start(u_out_r[oc][:, bsl], ust[:])

                def build_vt8(p, name):
                    vt8 = vt8p.tile([128, 2, 512], F8, tag="vt8", name=name)
                    for m in (0, 1):
                        tl = NT16 + 2 * p + m            # local k-tile index
                        il8 = tl // 3 - NI16              # index into hsb8
                        jb = tl % 3
                        nc.vector.tensor_mul(
                            vt8[:, m, :], htt_slice(bh, jb), hsb8_t[:, il8, bsl]
                        )
                    return vt8

                # fp8 DoubleRow pairs. The last weight batch runs oc-major
                # (all its pairs for one output chunk, then the next) so each
                # bank finishes, flushes, and DMAs out ~8us before the last
                # matmul — the tail exposes only one 256KB DMA, and the next
                # bh's banks are already free when its stream starts.
                if NP8:
                    p = 0
                    for pi, _ in enumerate(p_batches):
                        w8_t, p0, npb = w2_tiles[("f8", bh, pi)]
                        if pi < len(p_batches) - 1:
                            for pp in range(npb):
                                vt8 = build_vt8(p, f"vt8_{bh}_{p}")
                                for oc in range(6):
                                    nc.tensor.matmul(
                                        acc[oc][:],
                                        w8_t[:, pp, :, oc * 128 : (oc + 1) * 128],
                                        vt8[:],
                                        perf_mode=DR,
                                        start=False,
                                        stop=False,
                                    )
                                p += 1
                        else:
                            vt8s = [
                                build_vt8(p + pp, f"vt8e_{bh}_{pp}")
                                for pp in range(npb)
                            ]
                            for oc in range(6):
                                for pp in range(npb):
                                    nc.tensor.matmul(
                                        acc[oc][:],
                                        w8_t[:, pp, :, oc * 128 : (oc + 1) * 128],
                                        vt8s[pp][:],
                                        perf_mode=DR,
                                        start=False,
                                        stop=(pp == npb - 1),
                                    )
                                flush(oc, f"uste_{bh}_{oc}", last=(oc == 5))
                            p += npb
                else:
                    for oc in range(6):
                        flush(oc, f"ust_{bh}_{oc}")

    _split_excess_waits(nc)
    _strip_unused_mm_incs(nc)
    return nc


_NC_CACHE = None


def kernel(hspatial, htext, W_L, b_L, W_L2, b_L2):
    global LAST_EXEC_TIME_NS, LAST_RESULTS, _NC_CACHE

    hs = np.asarray(hspatial, dtype=np.float32)
    ht = np.asarray(htext, dtype=np.float32)
    W_L = np.asarray(W_L, dtype=np.float32)
    b_L = np.asarray(b_L, dtype=np.float32)
    W_L2 = np.asarray(W_L2, dtype=np.float32)
    b_L2 = np.asarray(b_L2, dtype=np.float32)

    htt = np.ascontiguousarray(ht.T.astype(NP_F16))        # [D, B]
    x = np.concatenate([hs, ht], axis=1)                   # [B, 2D]
    xt = np.ascontiguousarray(x.T.astype(NP_F16))          # [2D, B]
    wlt = np.ascontiguousarray(W_L.T.astype(NP_F16))       # [2D, 2D]

    in_maps = []
    for c in range(NCORES):
        sh0 = c * KC
        w2t_c = np.ascontiguousarray(
            W_L2[:, sh0 : sh0 + NT16 * 128].T.astype(NP_F16)
        )                                                   # [NT16*128, 2D]
        hs_c = hs[:, c * IC : c * IC + NI16].T.astype(NP_F16)   # [NI16, B]
        hsb16_c = np.ascontiguousarray(
            np.broadcast_to(hs_c[:, None, :], (max(NI16, 1), 128, B))
        )
        xt_c = np.ascontiguousarray(xt[:, c * 128 : (c + 1) * 128])
        m = {
            "w2t": w2t_c,
            "hsb16": hsb16_c,
            "htt": htt,
            "xt": xt_c,
            "wlt": wlt,
        }
        if NP8:
            W8 = (
                W_L2[:, sh0 + NT16 * 128 : sh0 + KC] * np.float32(W8SCALE)
            ).T.astype(NP_F8)                               # [NI8*D, 2D]
            w28_c = np.ascontiguousarray(
                W8.reshape(NP8, 2, 128, TWO_D)
                .transpose(0, 2, 1, 3)
                .reshape(NP8, 128, 2 * TWO_D)
            )
            hs8_c = (
                hs[:, c * IC + NI16 : (c + 1) * IC].T / np.float32(W8SCALE)
            ).astype(NP_F16)                                # [NI8, B]
            hsb8_c = np.ascontiguousarray(
                np.broadcast_to(hs8_c[:, None, :], (NI8, 128, B))
            )
            m["w28"] = w28_c
            m["hsb8"] = hsb8_c
        in_maps.append(m)

    if _NC_CACHE is None:
        _NC_CACHE = _gen()
    nc = _NC_CACHE

    res = run_bass_kernel_spmd(nc, in_maps, core_ids=list(range(NCORES)))
    LAST_EXEC_TIME_NS = res.exec_time_ns
    LAST_RESULTS = res

    ut = np.zeros((TWO_D, B), dtype=np.float64)
    for c in range(NCORES):
        ut += res.results[c]["u_out"]
    u = (ut.T + b_L2).astype(np.float32)

    g = np.concatenate([res.results[c]["g_out"] for c in range(NCORES)], axis=0)
    g = g + b_L

    return np.concatenate([g, u], axis=1).astype(np.float32)
